# revision 1
# baseline (speedup 1.0000x reference)
"""GCN message-passing kernel for Trainium2 (8 NeuronCores, Bass/Tile).

Computation (see reference):
  h   = relu(GCNConv(x, edge_index; w_gcn, b_gcn=0))   # sym-normalized A+I
  h   = relu(h @ w_lin + b_lin)
  out = h @ w_fc + b_fc

Sharding: nodes (segment targets) are split contiguously across the 8
cores (6250 each).  Each core recomputes the full transformed node table
h' = (dinv * x) @ w_gcn in fp16 (dinv[src] pre-folded into x on host),
then gathers h'[src] rows for its own edges with the SWDGE dma_gather
engine and segment-sums them on the PE with per-128-dst-window one-hot
matmuls whose one-hot values carry dinv[dst].  The MLP tail runs per
window with orientation-alternating matmuls so no transposes are needed.
"""

import sys

sys.path.insert(0, "/opt/trn_rl_repo")

import numpy as np

import concourse.bass as bass
import concourse.bacc as bacc
import concourse.tile as tile
import concourse.mybir as mybir
from concourse.bass_utils import run_bass_kernel_spmd
from concourse.library_config import mlp as mlp_lib

F16 = mybir.dt.float16
F32 = mybir.dt.float32
I16 = mybir.dt.int16
AF = mybir.ActivationFunctionType
OP = mybir.AluOpType

N = 50000
E = 600000
F_IN = 128
EMB = 128
F_OUT = 64
CORES = 8
NPC = N // CORES            # 6250 dst nodes per core
WSZ = 128                   # dst window (PSUM partition width)
NW = (NPC + WSZ - 1) // WSZ  # 49 windows per core
NT = (N + 127) // 128       # 391 node tiles
NP = NT * 128               # 50048 padded node count
HALF = 196 * 128            # 25088: tile-aligned gather-table split (int16 idx)
CH_BLK = 8                  # gather chunk = 8 blocks = 1024 edges (ucode cap ~1k idxs/call)

_CACHE = {}


def _build(nblk, nchunk, trace_label=""):
    """Build + compile the SPMD program.  nblk: [NW,2] int blocks per
    (window, half) group (uniform across cores); nchunk: [2] chunks/stream."""
    key = (tuple(nblk.ravel()), tuple(nchunk))
    if key in _CACHE:
        return _CACHE[key]

    blk_stream = [int(nblk[:, s].sum()) for s in range(2)]  # blocks per stream
    btot = blk_stream[0] + blk_stream[1]
    # base block index of group (w, s) within its stream
    base = np.zeros((NW, 2), np.int64)
    for s in range(2):
        base[:, s] = np.cumsum(nblk[:, s]) - nblk[:, s]
    # dcol/ddst column base of group (w,s): stream-hi columns after all lo
    colbase = base.copy()
    colbase[:, 1] += blk_stream[0]

    nc = bacc.Bacc("TRN2", debug=False)

    xsT_d = nc.dram_tensor("xsT", [F_IN, NP], F16, kind="ExternalInput")
    wgcn_d = nc.dram_tensor("wgcn", [F_IN, EMB], F16, kind="ExternalInput")
    wlin_d = nc.dram_tensor("wlin", [EMB, EMB], F16, kind="ExternalInput")
    wfc_d = nc.dram_tensor("wfc", [EMB, F_OUT], F16, kind="ExternalInput")
    blin_d = nc.dram_tensor("blin", [EMB, 1], F32, kind="ExternalInput")
    bfc_d = nc.dram_tensor("bfc", [128, F_OUT], F32, kind="ExternalInput")
    iota_d = nc.dram_tensor("iota", [128, WSZ], F16, kind="ExternalInput")
    dcol_d = nc.dram_tensor("dcol", [128, btot], F16, kind="ExternalInput")
    ddst_d = nc.dram_tensor("ddst", [128, btot], F16, kind="ExternalInput")
    gi_d = [
        nc.dram_tensor(f"gidx{s}", [max(nchunk[s], 1), 128, CH_BLK * 8], I16,
                       kind="ExternalInput")
        for s in range(2)
    ]
    out_d = nc.dram_tensor("out", [NW * WSZ, F_OUT], F32, kind="ExternalOutput")
    ht0_d = nc.dram_tensor("ht0", [HALF, EMB], F16)  # node table, lo half
    ht1_d = nc.dram_tensor("ht1", [NP - HALF, EMB], F16)  # node table, hi half

    with tile.TileContext(nc) as tc:
        with (
            tc.tile_pool(name="const", bufs=1) as cpool,
            tc.tile_pool(name="p1", bufs=6) as p1pool,
            tc.tile_pool(name="gbuf", bufs=6) as gpool,
            tc.tile_pool(name="sbld", bufs=4) as spool,
            tc.tile_pool(name="idx", bufs=4) as ipool,
            tc.tile_pool(name="mlp", bufs=4) as mpool,
            tc.tile_pool(name="psw", bufs=2, space="PSUM") as pswpool,
            tc.tile_pool(name="ps2", bufs=2, space="PSUM") as ps2pool,
            tc.tile_pool(name="ps3", bufs=1, space="PSUM") as ps3pool,
            tc.tile_pool(name="ps1", bufs=3, space="PSUM") as ps1pool,
        ):
            nc.gpsimd.load_library(mlp_lib)

            wgcn_s = cpool.tile([F_IN, EMB], F16)
            nc.sync.dma_start(wgcn_s[:], wgcn_d[:])
            wlin_s = cpool.tile([EMB, EMB], F16)
            nc.sync.dma_start(wlin_s[:], wlin_d[:])
            wfc_s = cpool.tile([EMB, F_OUT], F16)
            nc.sync.dma_start(wfc_s[:], wfc_d[:])
            blin_s = cpool.tile([EMB, 1], F32)
            nc.sync.dma_start(blin_s[:], blin_d[:])
            bfc_s = cpool.tile([128, F_OUT], F32)
            nc.sync.dma_start(bfc_s[:], bfc_d[:])
            iota_s = cpool.tile([128, WSZ], F16)
            nc.sync.dma_start(iota_s[:], iota_d[:])
            dcol_s = cpool.tile([128, btot], F16)
            nc.sync.dma_start(dcol_s[:], dcol_d[:])
            ddst_s = cpool.tile([128, btot], F16)
            nc.sync.dma_start(ddst_s[:], ddst_d[:])

            # ---- phase 1: h' = (dinv*x) @ w_gcn, fp16, to DRAM table ----
            # 4 node-tiles per iteration: one load DMA, 4 matmuls into one
            # PSUM bank, one fp16 copy, one store DMA.  lo half first so
            # lo gathers can start while the hi half still computes.
            assert NT % 4 == 3 and (NT + 1) % 4 == 0
            for t4 in range((NT + 1) // 4):
                nt_here = min(4, NT - t4 * 4)
                xt = p1pool.tile([F_IN, 4, 128], F16, tag="xt")
                nc.sync.dma_start(
                    xt[:, 0:nt_here, :],
                    xsT_d[:, t4 * 512 : t4 * 512 + nt_here * 128].rearrange(
                        "p (j f) -> p j f", f=128),
                )
                ps = ps1pool.tile([128, 4, 128], F32)
                for j in range(nt_here):
                    nc.tensor.matmul(ps[:, j, :], xt[:, j, :], wgcn_s[:],
                                     start=True, stop=True)
                hsb = p1pool.tile([128, 4, 128], F16, tag="hsb")
                nc.vector.tensor_copy(hsb[:, 0:nt_here, :], ps[:, 0:nt_here, :])
                r0 = t4 * 512
                dst = (ht0_d[r0 : r0 + nt_here * 128, :] if r0 < HALF
                       else ht1_d[r0 - HALF : r0 - HALF + nt_here * 128, :])
                nc.scalar.dma_start(
                    dst.rearrange("(j p) f -> p j f", p=128),
                    hsb[:, 0:nt_here, :],
                )

            # ---- phase 2: gather + windowed segment-sum + MLP tail ----
            ht_half = [ht0_d[:], ht1_d[:]]
            chunk_tiles = [dict(), dict()]

            def get_chunk(s, k):
                if k in chunk_tiles[s]:
                    return chunk_tiles[s][k]
                nb = min(CH_BLK, blk_stream[s] - k * CH_BLK)
                it = ipool.tile([128, CH_BLK * 8], I16, tag="idx")
                nc.sync.dma_start(it[:], gi_d[s][k, :, :])
                gt = gpool.tile([128, CH_BLK, EMB], F16, tag="g")
                nc.gpsimd.dma_gather(
                    gt[:, 0:nb, :],
                    ht_half[s][:],
                    it[:, 0 : nb * 8],
                    nb * 128,
                    nb * 128,
                    EMB,
                )
                chunk_tiles[s][k] = gt
                return gt

            for w in range(NW):
                psw = pswpool.tile([128, WSZ], F32)
                nblks_w = int(nblk[w, 0] + nblk[w, 1])
                assert nblks_w > 0
                bi = 0
                for s in range(2):
                    nb_g = int(nblk[w, s])
                    if nb_g == 0:
                        continue
                    # one-hot * dinv[dst] for this group's blocks
                    st = spool.tile([128, nb_g, WSZ], F16, tag="s")
                    c0 = int(colbase[w, s])
                    nc.vector.tensor_tensor(
                        st[:],
                        iota_s[:].unsqueeze(1).broadcast_to([128, nb_g, WSZ]),
                        dcol_s[:, c0 : c0 + nb_g]
                        .unsqueeze(2)
                        .broadcast_to([128, nb_g, WSZ]),
                        OP.is_equal,
                    )
                    nc.vector.tensor_tensor(
                        st[:],
                        st[:],
                        ddst_s[:, c0 : c0 + nb_g]
                        .unsqueeze(2)
                        .broadcast_to([128, nb_g, WSZ]),
                        OP.mult,
                    )
                    for j in range(nb_g):
                        q = int(base[w, s]) + j
                        gt = get_chunk(s, q // CH_BLK)
                        nc.tensor.matmul(
                            psw[:],
                            gt[:, q % CH_BLK, :],
                            st[:, j, :],
                            start=(bi == 0),
                            stop=(bi == nblks_w - 1),
                        )
                        bi += 1
                # h1T[f, d] = relu(psw)   (dinv[dst] already in S, b_gcn==0)
                h1t = mpool.tile([EMB, WSZ], F16, tag="h1t")
                nc.scalar.activation(h1t[:], psw[:], AF.Relu)
                # h2T[f2, d] = relu(w_lin.T @ h1 + b_lin)
                ps2 = ps2pool.tile([EMB, WSZ], F32)
                nc.tensor.matmul(ps2[:], wlin_s[:], h1t[:], start=True, stop=True)
                h2t = mpool.tile([EMB, WSZ], F16, tag="h2t")
                nc.scalar.activation(h2t[:], ps2[:], AF.Relu, bias=blin_s[:, 0:1])
                # out[d, f_out] = h2 @ w_fc + b_fc
                ps3 = ps3pool.tile([WSZ, F_OUT], F32)
                nc.tensor.matmul(ps3[:], h2t[:], wfc_s[:], start=True, stop=True)
                osb = mpool.tile([WSZ, F_OUT], F32, tag="osb")
                nc.vector.tensor_tensor(osb[:], ps3[:], bfc_s[:], OP.add)
                nc.scalar.dma_start(out_d[bass.ts(w, WSZ), :], osb[:])

    nc.compile()
    _CACHE[key] = (nc, blk_stream)
    return _CACHE[key]


def _wrap_idx(a):
    """int16 [n*16k...] -> dma_gather idx layout [128, n/16] (16-part wrap,
    replicated 8x across the 128 partitions)."""
    w = a.reshape(-1, 16).T  # [16, n/16]
    return np.tile(w, (8, 1)).astype(np.int16)


def kernel(x, edge_index, w_gcn, b_gcn, w_lin, b_lin, w_fc, b_fc, _trace=False):
    x = np.asarray(x, np.float32)
    edge_index = np.asarray(edge_index)
    assert np.max(np.abs(np.asarray(b_gcn))) == 0.0, "b_gcn expected zero"

    src = edge_index[0].astype(np.int64)
    dst = edge_index[1].astype(np.int64)
    loop = np.arange(N, dtype=np.int64)
    src = np.concatenate([src, loop])
    dst = np.concatenate([dst, loop])

    deg = np.bincount(dst, minlength=N).astype(np.float32)
    dinv = 1.0 / np.sqrt(deg)

    # fold dinv[src] into x; transpose + fp16 for the device
    xsT = np.zeros((F_IN, NP), np.float16)
    xsT[:, :N] = (x * dinv[:, None]).T.astype(np.float16)

    core = dst // NPC
    dl = dst - core * NPC
    win = dl >> 7
    drel = dl & 127
    half = (src >= HALF).astype(np.int64)
    gidx = (src - half * HALF).astype(np.int64)

    cnt = np.zeros((CORES, NW, 2), np.int64)
    np.add.at(cnt, (core, win, half), 1)
    nblk = ((cnt.max(axis=0) + 127) // 128).astype(np.int64)  # [NW, 2]
    blk_stream = [int(nblk[:, s].sum()) for s in range(2)]
    btot = blk_stream[0] + blk_stream[1]
    nchunk = [max(1, (bs + CH_BLK - 1) // CH_BLK) for bs in blk_stream]

    base = np.zeros((NW, 2), np.int64)
    for s in range(2):
        base[:, s] = np.cumsum(nblk[:, s]) - nblk[:, s]
    colbase = base.copy()
    colbase[:, 1] += blk_stream[0]

    # sort edges by (core, win, half); stable order within groups
    key = ((core * NW + win) * 2 + half)
    perm = np.argsort(key, kind="stable")
    gidx_s, win_s, drel_s, half_s, core_s = (
        gidx[perm], win[perm], drel[perm], half[perm], core[perm])
    dinv_dst_s = dinv[dst[perm]].astype(np.float32)

    grp_cnt = cnt  # [CORES, NW, 2]
    # per-core padded streams
    in_maps = []
    for c in range(CORES):
        gstream = [np.zeros(max(nchunk[s], 1) * CH_BLK * 128, np.int16)
                   for s in range(2)]
        dcol = np.full((btot * 128,), -1.0, np.float16)
        ddst = np.zeros((btot * 128,), np.float16)
        # edge range of this core in the sorted arrays
        e0 = np.searchsorted(core_s, c)
        e1 = np.searchsorted(core_s, c + 1)
        off = e0
        for w in range(NW):
            for s in range(2):
                n_e = int(grp_cnt[c, w, s])
                if n_e:
                    sl = slice(off, off + n_e)
                    p0 = int(base[w, s]) * 128
                    gstream[s][p0 : p0 + n_e] = gidx_s[sl]
                    q0 = int(colbase[w, s]) * 128
                    dcol[q0 : q0 + n_e] = drel_s[sl].astype(np.float16)
                    ddst[q0 : q0 + n_e] = dinv_dst_s[sl].astype(np.float16)
                    off += n_e
        assert off == e1
        im = {
            "xsT": xsT,
            "wgcn": np.asarray(w_gcn, np.float32).astype(np.float16),
            "wlin": np.asarray(w_lin, np.float32).astype(np.float16),
            "wfc": np.asarray(w_fc, np.float32).astype(np.float16),
            "blin": np.asarray(b_lin, np.float32).reshape(EMB, 1),
            "bfc": np.tile(np.asarray(b_fc, np.float32).reshape(1, F_OUT),
                           (128, 1)),
            "iota": np.tile(np.arange(WSZ, dtype=np.float16).reshape(1, WSZ),
                            (128, 1)),
            "dcol": dcol.reshape(btot, 128).T.copy(),
            "ddst": ddst.reshape(btot, 128).T.copy(),
        }
        for s in range(2):
            wrapped = _wrap_idx(gstream[s])  # [128, tot/16]
            im[f"gidx{s}"] = np.ascontiguousarray(
                wrapped.reshape(128, max(nchunk[s], 1), CH_BLK * 8)
                .transpose(1, 0, 2))
        in_maps.append(im)

    nc, _ = _build(nblk, nchunk)
    res = run_bass_kernel_spmd(nc, in_maps, list(range(CORES)), trace=_trace)

    out = np.empty((N, F_OUT), np.float32)
    for c in range(CORES):
        out[c * NPC : (c + 1) * NPC] = res.results[c]["out"][:NPC]
    kernel._last_results = res
    return out



# revision 3
# speedup vs baseline: 1.5689x; 1.5689x over previous
"""GCN message-passing kernel for Trainium2 (8 NeuronCores, Bass/Tile).

Computation (see reference):
  h   = relu(GCNConv(x, edge_index; w_gcn, b_gcn=0))   # sym-normalized A+I
  h   = relu(h @ w_lin + b_lin)
  out = h @ w_fc + b_fc

Sharding: nodes (segment targets) are split contiguously across the 8
cores (6250 each).  Each core recomputes the full transformed node table
h' = (dinv * x) @ w_gcn in fp16 (dinv[src] pre-folded into x on host),
then gathers h'[src] rows for its own edges with the SWDGE dma_gather
engine and segment-sums them on the PE with per-128-dst-window one-hot
matmuls whose one-hot values carry dinv[dst].  The MLP tail runs per
window with orientation-alternating matmuls so no transposes are needed.
"""

import sys

sys.path.insert(0, "/opt/trn_rl_repo")

import numpy as np

import concourse.bass as bass
import concourse.bacc as bacc
import concourse.tile as tile
import concourse.mybir as mybir
from concourse.bass_utils import run_bass_kernel_spmd
from concourse.library_config import mlp as mlp_lib

F16 = mybir.dt.float16
F32 = mybir.dt.float32
I16 = mybir.dt.int16
AF = mybir.ActivationFunctionType
OP = mybir.AluOpType

N = 50000
E = 600000
F_IN = 128
EMB = 128
F_OUT = 64
CORES = 8
NPC = N // CORES            # 6250 dst nodes per core
WSZ = 128                   # dst window (PSUM partition width)
NW = (NPC + WSZ - 1) // WSZ  # 49 windows per core
NT = (N + 127) // 128       # 391 node tiles
NP = NT * 128               # 50048 padded node count
HALF = 196 * 128            # 25088: tile-aligned gather-table split (int16 idx)
CH_BLK = 8                  # gather chunk = 8 blocks = 1024 edges (ucode cap ~1k idxs/call)

_CACHE = {}


def _build(nblk, nchunk, trace_label=""):
    """Build + compile the SPMD program.  nblk: [NW,2] int blocks per
    (window, half) group (uniform across cores); nchunk: [2] chunks/stream."""
    key = (tuple(nblk.ravel()), tuple(nchunk))
    if key in _CACHE:
        return _CACHE[key]

    blk_stream = [int(nblk[:, s].sum()) for s in range(2)]  # blocks per stream
    btot = blk_stream[0] + blk_stream[1]
    # base block index of group (w, s) within its stream
    base = np.zeros((NW, 2), np.int64)
    for s in range(2):
        base[:, s] = np.cumsum(nblk[:, s]) - nblk[:, s]
    # dcol/ddst column base of group (w,s): stream-hi columns after all lo
    colbase = base.copy()
    colbase[:, 1] += blk_stream[0]

    nc = bacc.Bacc("TRN2", debug=False, num_swdge_queues=4,
                   dynamic_dma_scratch_size=65536)

    xsT_d = nc.dram_tensor("xsT", [F_IN, NP], F16, kind="ExternalInput")
    wgcn_d = nc.dram_tensor("wgcn", [F_IN, EMB], F16, kind="ExternalInput")
    wlin_d = nc.dram_tensor("wlin", [EMB, EMB], F16, kind="ExternalInput")
    wfc_d = nc.dram_tensor("wfc", [EMB, F_OUT], F16, kind="ExternalInput")
    blin_d = nc.dram_tensor("blin", [EMB, 1], F32, kind="ExternalInput")
    bfc_d = nc.dram_tensor("bfc", [128, F_OUT], F32, kind="ExternalInput")
    iota_d = nc.dram_tensor("iota", [128, WSZ], F16, kind="ExternalInput")
    dcol_d = nc.dram_tensor("dcol", [128, btot], F16, kind="ExternalInput")
    ddst_d = nc.dram_tensor("ddst", [128, btot], F16, kind="ExternalInput")
    gi_d = [
        nc.dram_tensor(f"gidx{s}", [max(nchunk[s], 1), 128, CH_BLK * 8], I16,
                       kind="ExternalInput")
        for s in range(2)
    ]
    out_d = nc.dram_tensor("out", [NW * WSZ, F_OUT], F32, kind="ExternalOutput")
    ht0_d = nc.dram_tensor("ht0", [HALF, EMB], F16)  # node table, lo half
    ht1_d = nc.dram_tensor("ht1", [NP - HALF, EMB], F16)  # node table, hi half

    with tile.TileContext(nc) as tc:
        with (
            tc.tile_pool(name="const", bufs=1) as cpool,
            tc.tile_pool(name="p1", bufs=6) as p1pool,
            tc.tile_pool(name="gbuf", bufs=6) as gpool,
            tc.tile_pool(name="sbld", bufs=4) as spool,
            tc.tile_pool(name="idx", bufs=4) as ipool,
            tc.tile_pool(name="mlp", bufs=4) as mpool,
            tc.tile_pool(name="psw", bufs=2, space="PSUM") as pswpool,
            tc.tile_pool(name="ps2", bufs=2, space="PSUM") as ps2pool,
            tc.tile_pool(name="ps3", bufs=1, space="PSUM") as ps3pool,
            tc.tile_pool(name="ps1", bufs=3, space="PSUM") as ps1pool,
        ):
            nc.gpsimd.load_library(mlp_lib)

            wgcn_s = cpool.tile([F_IN, EMB], F16)
            nc.sync.dma_start(wgcn_s[:], wgcn_d[:])
            wlin_s = cpool.tile([EMB, EMB], F16)
            nc.sync.dma_start(wlin_s[:], wlin_d[:])
            wfc_s = cpool.tile([EMB, F_OUT], F16)
            nc.sync.dma_start(wfc_s[:], wfc_d[:])
            blin_s = cpool.tile([EMB, 1], F32)
            nc.sync.dma_start(blin_s[:], blin_d[:])
            bfc_s = cpool.tile([128, F_OUT], F32)
            nc.sync.dma_start(bfc_s[:], bfc_d[:])
            iota_s = cpool.tile([128, WSZ], F16)
            nc.sync.dma_start(iota_s[:], iota_d[:])
            dcol_s = cpool.tile([128, btot], F16)
            nc.sync.dma_start(dcol_s[:], dcol_d[:])
            ddst_s = cpool.tile([128, btot], F16)
            nc.sync.dma_start(ddst_s[:], ddst_d[:])

            # ---- phase 1: h' = (dinv*x) @ w_gcn, fp16, to DRAM table ----
            # 4 node-tiles per iteration: one load DMA, 4 matmuls into one
            # PSUM bank, one fp16 copy, one store DMA.  lo half first so
            # lo gathers can start while the hi half still computes.
            assert NT % 4 == 3 and (NT + 1) % 4 == 0
            for t4 in range((NT + 1) // 4):
                nt_here = min(4, NT - t4 * 4)
                xt = p1pool.tile([F_IN, 4, 128], F16, tag="xt")
                nc.sync.dma_start(
                    xt[:, 0:nt_here, :],
                    xsT_d[:, t4 * 512 : t4 * 512 + nt_here * 128].rearrange(
                        "p (j f) -> p j f", f=128),
                )
                ps = ps1pool.tile([128, 4, 128], F32)
                for j in range(nt_here):
                    nc.tensor.matmul(ps[:, j, :], xt[:, j, :], wgcn_s[:],
                                     start=True, stop=True)
                hsb = p1pool.tile([128, 4, 128], F16, tag="hsb")
                nc.vector.tensor_copy(hsb[:, 0:nt_here, :], ps[:, 0:nt_here, :])
                r0 = t4 * 512
                dst = (ht0_d[r0 : r0 + nt_here * 128, :] if r0 < HALF
                       else ht1_d[r0 - HALF : r0 - HALF + nt_here * 128, :])
                nc.scalar.dma_start(
                    dst.rearrange("(j p) f -> p j f", p=128),
                    hsb[:, 0:nt_here, :],
                )

            # ---- phase 2: gather + windowed segment-sum + MLP tail ----
            ht_half = [ht0_d[:], ht1_d[:]]
            chunk_tiles = [dict(), dict()]
            qctr = [0]

            def get_chunk(s, k):
                if k in chunk_tiles[s]:
                    return chunk_tiles[s][k]
                nb = min(CH_BLK, blk_stream[s] - k * CH_BLK)
                it = ipool.tile([128, CH_BLK * 8], I16, tag="idx")
                nc.sync.dma_start(it[:], gi_d[s][k, :, :])
                gt = gpool.tile([128, CH_BLK, EMB], F16, tag="g")
                nc.gpsimd.dma_gather(
                    gt[:, 0:nb, :],
                    ht_half[s][:],
                    it[:, 0 : nb * 8],
                    nb * 128,
                    nb * 128,
                    EMB,
                    queue_num=qctr[0] % 4,
                )
                qctr[0] += 1
                chunk_tiles[s][k] = gt
                return gt

            for w in range(NW):
                psw = pswpool.tile([128, WSZ], F32)
                nblks_w = int(nblk[w, 0] + nblk[w, 1])
                assert nblks_w > 0
                bi = 0
                for s in range(2):
                    nb_g = int(nblk[w, s])
                    if nb_g == 0:
                        continue
                    # one-hot * dinv[dst] for this group's blocks
                    st = spool.tile([128, nb_g, WSZ], F16, tag="s")
                    c0 = int(colbase[w, s])
                    nc.vector.tensor_tensor(
                        st[:],
                        iota_s[:].unsqueeze(1).broadcast_to([128, nb_g, WSZ]),
                        dcol_s[:, c0 : c0 + nb_g]
                        .unsqueeze(2)
                        .broadcast_to([128, nb_g, WSZ]),
                        OP.is_equal,
                    )
                    nc.vector.tensor_tensor(
                        st[:],
                        st[:],
                        ddst_s[:, c0 : c0 + nb_g]
                        .unsqueeze(2)
                        .broadcast_to([128, nb_g, WSZ]),
                        OP.mult,
                    )
                    for j in range(nb_g):
                        q = int(base[w, s]) + j
                        gt = get_chunk(s, q // CH_BLK)
                        nc.tensor.matmul(
                            psw[:],
                            gt[:, q % CH_BLK, :],
                            st[:, j, :],
                            start=(bi == 0),
                            stop=(bi == nblks_w - 1),
                        )
                        bi += 1
                # h1T[f, d] = relu(psw)   (dinv[dst] already in S, b_gcn==0)
                h1t = mpool.tile([EMB, WSZ], F16, tag="h1t")
                nc.scalar.activation(h1t[:], psw[:], AF.Relu)
                # h2T[f2, d] = relu(w_lin.T @ h1 + b_lin)
                ps2 = ps2pool.tile([EMB, WSZ], F32)
                nc.tensor.matmul(ps2[:], wlin_s[:], h1t[:], start=True, stop=True)
                h2t = mpool.tile([EMB, WSZ], F16, tag="h2t")
                nc.scalar.activation(h2t[:], ps2[:], AF.Relu, bias=blin_s[:, 0:1])
                # out[d, f_out] = h2 @ w_fc + b_fc
                ps3 = ps3pool.tile([WSZ, F_OUT], F32)
                nc.tensor.matmul(ps3[:], h2t[:], wfc_s[:], start=True, stop=True)
                osb = mpool.tile([WSZ, F_OUT], F32, tag="osb")
                nc.vector.tensor_tensor(osb[:], ps3[:], bfc_s[:], OP.add)
                nc.scalar.dma_start(out_d[bass.ts(w, WSZ), :], osb[:])

    nc.compile()
    _CACHE[key] = (nc, blk_stream)
    return _CACHE[key]


def _wrap_idx(a):
    """int16 [n*16k...] -> dma_gather idx layout [128, n/16] (16-part wrap,
    replicated 8x across the 128 partitions)."""
    w = a.reshape(-1, 16).T  # [16, n/16]
    return np.tile(w, (8, 1)).astype(np.int16)


def kernel(x, edge_index, w_gcn, b_gcn, w_lin, b_lin, w_fc, b_fc, _trace=False):
    x = np.asarray(x, np.float32)
    edge_index = np.asarray(edge_index)
    assert np.max(np.abs(np.asarray(b_gcn))) == 0.0, "b_gcn expected zero"

    src = edge_index[0].astype(np.int64)
    dst = edge_index[1].astype(np.int64)
    loop = np.arange(N, dtype=np.int64)
    src = np.concatenate([src, loop])
    dst = np.concatenate([dst, loop])

    deg = np.bincount(dst, minlength=N).astype(np.float32)
    dinv = 1.0 / np.sqrt(deg)

    # fold dinv[src] into x; transpose + fp16 for the device
    xsT = np.zeros((F_IN, NP), np.float16)
    xsT[:, :N] = (x * dinv[:, None]).T.astype(np.float16)

    core = dst // NPC
    dl = dst - core * NPC
    win = dl >> 7
    drel = dl & 127
    half = (src >= HALF).astype(np.int64)
    gidx = (src - half * HALF).astype(np.int64)

    cnt = np.zeros((CORES, NW, 2), np.int64)
    np.add.at(cnt, (core, win, half), 1)
    nblk = ((cnt.max(axis=0) + 127) // 128).astype(np.int64)  # [NW, 2]
    blk_stream = [int(nblk[:, s].sum()) for s in range(2)]
    btot = blk_stream[0] + blk_stream[1]
    nchunk = [max(1, (bs + CH_BLK - 1) // CH_BLK) for bs in blk_stream]

    base = np.zeros((NW, 2), np.int64)
    for s in range(2):
        base[:, s] = np.cumsum(nblk[:, s]) - nblk[:, s]
    colbase = base.copy()
    colbase[:, 1] += blk_stream[0]

    # sort edges by (core, win, half); stable order within groups
    key = ((core * NW + win) * 2 + half)
    perm = np.argsort(key, kind="stable")
    gidx_s, win_s, drel_s, half_s, core_s = (
        gidx[perm], win[perm], drel[perm], half[perm], core[perm])
    dinv_dst_s = dinv[dst[perm]].astype(np.float32)

    grp_cnt = cnt  # [CORES, NW, 2]
    # per-core padded streams
    in_maps = []
    for c in range(CORES):
        gstream = [np.zeros(max(nchunk[s], 1) * CH_BLK * 128, np.int16)
                   for s in range(2)]
        dcol = np.full((btot * 128,), -1.0, np.float16)
        ddst = np.zeros((btot * 128,), np.float16)
        # edge range of this core in the sorted arrays
        e0 = np.searchsorted(core_s, c)
        e1 = np.searchsorted(core_s, c + 1)
        off = e0
        for w in range(NW):
            for s in range(2):
                n_e = int(grp_cnt[c, w, s])
                if n_e:
                    sl = slice(off, off + n_e)
                    p0 = int(base[w, s]) * 128
                    gstream[s][p0 : p0 + n_e] = gidx_s[sl]
                    q0 = int(colbase[w, s]) * 128
                    dcol[q0 : q0 + n_e] = drel_s[sl].astype(np.float16)
                    ddst[q0 : q0 + n_e] = dinv_dst_s[sl].astype(np.float16)
                    off += n_e
        assert off == e1
        im = {
            "xsT": xsT,
            "wgcn": np.asarray(w_gcn, np.float32).astype(np.float16),
            "wlin": np.asarray(w_lin, np.float32).astype(np.float16),
            "wfc": np.asarray(w_fc, np.float32).astype(np.float16),
            "blin": np.asarray(b_lin, np.float32).reshape(EMB, 1),
            "bfc": np.tile(np.asarray(b_fc, np.float32).reshape(1, F_OUT),
                           (128, 1)),
            "iota": np.tile(np.arange(WSZ, dtype=np.float16).reshape(1, WSZ),
                            (128, 1)),
            "dcol": dcol.reshape(btot, 128).T.copy(),
            "ddst": ddst.reshape(btot, 128).T.copy(),
        }
        for s in range(2):
            wrapped = _wrap_idx(gstream[s])  # [128, tot/16]
            im[f"gidx{s}"] = np.ascontiguousarray(
                wrapped.reshape(128, max(nchunk[s], 1), CH_BLK * 8)
                .transpose(1, 0, 2))
        in_maps.append(im)

    nc, _ = _build(nblk, nchunk)
    res = run_bass_kernel_spmd(nc, in_maps, list(range(CORES)), trace=_trace)

    out = np.empty((N, F_OUT), np.float32)
    for c in range(CORES):
        out[c * NPC : (c + 1) * NPC] = res.results[c]["out"][:NPC]
    kernel._last_results = res
    return out

